# revision 9
# baseline (speedup 1.0000x reference)
"""Trainium2 Bass kernel for nn_CircularBlur: depthwise 4x4 blur with
circular padding on (4, 512, 256, 256) fp32.

Math (derived from the reference's wrap-pad + zero-pad + flipped-kernel
conv + crop; the zero padding never reaches the cropped region):

    out[n,c,y,x] = sum_{i,j} k[i,j] * in[n,c,(y+1-i)%256,(x+1-j)%256]

Strategy: pure data parallel over the 2048 (n,c) images, 256 per core.
Per image the blur is separable (k = a outer b via SVD).  The vertical
pass is a banded-circulant matmul on the tensor engine (stationary =
128x128 chunks of V^T, prescaled by the horizontal tap weights).  The
horizontal taps become shifted column windows of the moving operand;
symmetric tap pairs are pre-summed element-wise so each pair costs one
matmul instead of two.  Column wrap is handled with on-chip boundary-
column ops; row wrap is baked into V.

The kernel is HBM-bound.  The rel-err budget (2e-2) is spent on an
fp16 input path: the host ships x as fp16, halving read traffic
(128 -> 96 MiB per core, ~270 us floor).  The output stays fp32.
Engine balance at that floor:
  - loads on the Sync HWDGE queue, stores + weight load on the ACT
    HWDGE queue: a store waiting on compute never head-of-line-blocks
    load descriptor generation, and the SDMA engines round-robin
    between the two rings.
  - the two moving-tensor builds run on Vector and GpSimd in parallel
    (~145 us each instead of ~290 us on Vector alone).  None of the
    element-wise ops are DVE perf-mode ops, so the DVE/GpSimd shared
    SBUF port-pair lock is never taken.
  - work is chunked at half-group granularity (4 images per DMA) with
    separate tiles per half: short ramp, small tail, smooth pipeline.
"""

import sys

sys.path.insert(0, "/opt/trn_rl_repo")

import numpy as np

N_CORES = 8
H = W = 256
IMG_TOTAL = 4 * 512
IMG_PER_CORE = IMG_TOTAL // N_CORES  # 256
G = 8   # images per group
HG = 4  # images per half-group (one DMA)
NGROUPS = IMG_PER_CORE // G
KH = KW = 4


def _decompose(k):
    """k (4,4) float64 -> list of rank-1 terms [(a[4], taps)], where
    taps = [(scale, [shifts...])]; shift s means column x+s contributes
    with weight scale (after pre-summing all shifts in the group)."""
    U, S, Vt = np.linalg.svd(k)
    terms = []
    for r in range(KH):
        if S[r] <= max(S[0] * 1e-7, 1e-30):
            continue
        a = U[:, r] * np.sqrt(S[r])
        b = Vt[r] * np.sqrt(S[r])
        # tap j has shift 1-j and weight b[j]
        tol = 1e-9 * max(1.0, np.abs(b).max())
        if abs(b[0] - b[3]) <= tol and abs(b[1] - b[2]) <= tol:
            taps = [(b[0], [1, -2]), (b[1], [0, -1])]
        else:
            taps = [(b[j], [1 - j]) for j in range(KW)]
        terms.append((a, taps))
    return terms


def _build_weights(terms):
    """Host-side stationary blocks.  Returns (W_host [128, NIDX, 128] f16,
    mov_shifts: list of shift-lists, one per moving tensor)."""
    movs = []  # (a_vec, scale, shifts)
    for a, taps in terms:
        for scale, shifts in taps:
            movs.append((a, scale, shifts))
    n_idx = len(movs) * 4
    Wh = np.zeros((128, n_idx, 128), np.float32)
    yy = np.arange(H)
    for mi, (a, scale, _shifts) in enumerate(movs):
        V = np.zeros((H, H), np.float64)
        for i in range(KH):
            V[yy, (yy + 1 - i) % H] += a[i]
        VT = (scale * V).T  # VT[v, y]
        for kc in range(2):
            for yb in range(2):
                idx = (mi * 2 + kc) * 2 + yb
                # row v=2*vp+kc lives on partition vp; out row y=2*m+yb on
                # psum partition m (even/odd interleave -> 2KB DMA chunks)
                Wh[:, idx, :] = VT[kc::2, yb::2].astype(np.float32)
    return Wh.astype(np.float16), [m[2] for m in movs]


_PROGRAM_CACHE = {}


def _build_program(mov_shifts):
    """Build + compile the per-core Bass program.  mov_shifts: list of
    shift-lists (structure only; weights arrive via the `w` input)."""
    import concourse.bacc as bacc
    import concourse.mybir as mybir
    from concourse import tile

    key = tuple(tuple(s) for s in mov_shifts)
    if key in _PROGRAM_CACHE:
        return _PROGRAM_CACHE[key]

    f32 = mybir.dt.float32
    f16 = mybir.dt.float16
    n_movs = len(mov_shifts)
    n_idx = n_movs * 4

    nc = bacc.Bacc("TRN2", target_bir_lowering=False, debug=False,
                   num_devices=N_CORES)
    x_in = nc.declare_dram_parameter("x", [IMG_PER_CORE, H, W], f16,
                                     isOutput=False)
    w_in = nc.declare_dram_parameter("w", [128, n_idx, 128], f16,
                                     isOutput=False)
    y_out = nc.declare_dram_parameter("y", [IMG_PER_CORE, H, W], f32,
                                      isOutput=True)

    # halo layout: xc col c holds image col x = c - 2 for c in [2, 258);
    # cols 0,1,258 are circular-wrap copies, so every shifted window
    # (shifts in [-2, +1]) is contiguous and in-range
    HW_ = W + 3

    def fill_halo(eng, xc):
        for c, src in ((0, 256), (1, 257), (258, 2)):
            eng.tensor_copy(xc[:, :, :, c:c + 1], xc[:, :, :, src:src + 1])

    def build_mov(eng, pt, xc, shifts):
        """pt[x] = sum_s xc[(x+s) % W], one full-width op (halo covers
        the wrap)."""
        if len(shifts) == 1:
            s = shifts[0] + 2
            eng.tensor_copy(pt[:], xc[:, :, :, s:s + W])
        else:
            assert len(shifts) == 2
            s0, s1 = shifts[0] + 2, shifts[1] + 2
            eng.tensor_add(
                pt[:], xc[:, :, :, s0:s0 + W], xc[:, :, :, s1:s1 + W]
            )

    with tile.TileContext(nc) as tc:
        with (
            tc.tile_pool(name="const", bufs=1) as cpool,
            tc.tile_pool(name="xin", bufs=6) as xpool,
            tc.tile_pool(name="mov", bufs=3) as mpool,
            tc.tile_pool(name="outp", bufs=4) as opool,
            tc.tile_pool(name="psum", bufs=6, space="PSUM") as pspool,
        ):
            # weight load on the ACT (scalar) queue so the sync queue's
            # first x-load descriptor generation starts immediately; one
            # DMA per idx block so the per-partition descriptors spread
            # over all 16 SDMA engines instead of landing on one
            wt = cpool.tile([128, n_idx, 128], f16)
            for i in range(n_idx):
                nc.scalar.dma_start(wt[:, i, :], w_in[:, i, :])

            # mov build engines: split across Vector and GpSimd (no DVE
            # perf-mode ops in this kernel, so no shared-port-pair lock)
            mov_eng = [nc.vector if ti % 2 == 0 else nc.gpsimd
                       for ti in range(n_movs)]

            for g in range(NGROUPS):
                # every 8th group skips the pre-sum and feeds shifted halo
                # windows straight to the tensor engine (2 MMs per tap
                # pair instead of 1): moves ~12.5% of the element-wise
                # work onto the tensor engine's slack so Vector, GpSimd,
                # Tensor and DMA all sit near the same ~270us busy
                direct = (g % 8 == 3)
                movs = [[None] * 2 for _ in range(n_movs)]
                xcs = [None, None]
                yts = [None, None]
                for h in range(2):
                    hs = slice(g * G + h * HG, g * G + (h + 1) * HG)
                    # per partition each image row is one contiguous 512B
                    # descriptor
                    xc = xpool.tile([128, HG, 2, HW_], f16, tag=f"xc{h}",
                                    name=f"xc{h}")
                    # one DMA per row-parity: the padded (stride-259) tile
                    # only AP-balances as [128][img][256] per r slice; the
                    # 2-per-half granularity also ramps the SDMA engines
                    # quickly at the head
                    src = x_in[hs].rearrange("m (p r) w -> p m r w", r=2)
                    for rr in range(2):
                        nc.sync.dma_start(
                            xc[:, :, rr, 2:2 + W], src[:, :, rr, :]
                        )
                    heng = nc.vector if h == 0 else nc.gpsimd
                    fill_halo(heng, xc)
                    xcs[h] = xc
                    if not direct:
                        for ti, shifts in enumerate(mov_shifts):
                            pt = mpool.tile([128, HG, 2, W], f16,
                                            tag=f"p{ti}h{h}",
                                            name=f"p{ti}h{h}")
                            build_mov(mov_eng[ti], pt, xc, shifts)
                            movs[ti][h] = pt
                    yts[h] = opool.tile([128, HG, 2, W], f32, tag=f"yt{h}",
                                        name=f"yt{h}")

                for pr in range(G // 2):
                    h, j = pr // 2, (pr % 2) * 2
                    yt = yts[h]
                    for yb in range(2):
                        ps = pspool.tile([128, 2, W], f32, tag="ps")
                        if direct:
                            mms = [(mi, kc, s) for mi in range(n_movs)
                                   for kc in range(2)
                                   for s in mov_shifts[mi]]
                        else:
                            mms = [(mi, kc, None) for mi in range(n_movs)
                                   for kc in range(2)]
                        for q, (mi, kc, s) in enumerate(mms):
                            idx = (mi * 2 + kc) * 2 + yb
                            if s is None:
                                rhs = movs[mi][h][:, j:j + 2, kc, 0:W]
                            else:
                                c0 = s + 2
                                rhs = xcs[h][:, j:j + 2, kc, c0:c0 + W]
                            nc.tensor.matmul(
                                ps[:], wt[:, idx, :], rhs,
                                start=(q == 0), stop=(q == len(mms) - 1),
                            )
                        nc.scalar.copy(yt[:, j:j + 2, yb, :], ps[:])
                    if pr % 2 == 1:
                        # store issued from the ACT queue right after its
                        # producing copies (program order, no cross-engine
                        # wait); separate ring from the loads
                        hs = slice(g * G + h * HG, g * G + (h + 1) * HG)
                        nc.scalar.dma_start(
                            y_out[hs].rearrange("m (p r) w -> p m r w", r=2),
                            yt[:],
                        )

    nc.compile()
    _PROGRAM_CACHE[key] = nc
    return nc


def _make_in_maps(input_np, Wh):
    """Per-core input maps.  The host ships x as fp16 (the rel-err budget
    is 2e-2; fp16 quantization costs ~5e-4) to halve HBM read traffic."""
    x16 = np.ascontiguousarray(
        np.asarray(input_np).reshape(IMG_TOTAL, H, W).astype(np.float16)
    )
    return [
        {"x": x16[c * IMG_PER_CORE:(c + 1) * IMG_PER_CORE], "w": Wh}
        for c in range(N_CORES)
    ]


def kernel(input, kernel):
    input = np.asarray(input, dtype=np.float32)
    k = np.asarray(kernel, dtype=np.float64)
    assert input.shape == (4, 512, H, W) and k.shape == (KH, KW)

    terms = _decompose(k)
    if not terms:
        return np.zeros_like(input)

    Wh, mov_shifts = _build_weights(terms)
    nc = _build_program(mov_shifts)

    from concourse.bass_utils import run_bass_kernel_spmd

    in_maps = _make_in_maps(input, Wh)
    res = run_bass_kernel_spmd(nc, in_maps, list(range(N_CORES)))
    out = np.concatenate([res.results[c]["y"] for c in range(N_CORES)], axis=0)
    return out.reshape(4, 512, H, W).astype(np.float32, copy=False)


# revision 10
# speedup vs baseline: 1.5191x; 1.5191x over previous
"""Trainium2 Bass kernel for nn_CircularBlur: depthwise 4x4 blur with
circular padding on (4, 512, 256, 256) fp32.

Math (derived from the reference's wrap-pad + zero-pad + flipped-kernel
conv + crop; the zero padding never reaches the cropped region):

    out[n,c,y,x] = sum_{i,j} k[i,j] * in[n,c,(y+1-i)%256,(x+1-j)%256]

Strategy: pure data parallel over the 2048 (n,c) images, 256 per core.
Per image the blur is separable (k = a outer b via SVD).  The vertical
pass is a banded-circulant matmul on the tensor engine (stationary =
128x128 chunks of V^T, prescaled by the horizontal tap weights).  The
horizontal taps become shifted column windows of the moving operand;
symmetric tap pairs are pre-summed on the vector engine so each pair
costs one matmul instead of two.  Column wrap is handled with a 3-col
halo in the on-chip tile; row wrap is baked into V.

The rel-err budget (2e-2) is spent on fp16 I/O: the host ships x as
fp16 and reads y back as fp16 (~3e-4 quantization), cutting HBM
traffic from 128 to 64 MiB per core.  Engine balance:
  - all element-wise work runs on the Vector engine.  With full-256
    wide halo windows the fp16 adds hit the DVE 2-elem/cycle fast path
    (~1.1us per half-group); GpSimd is kept fully idle so DVE never
    loses the shared SBUF port-pair arbitration.
  - the tensor engine (fp16 matmuls + fast weight load, ~237us) is the
    expected bottleneck; loads, stores, copies and descriptor
    generation are spread across Sync/ACT/DVE so nothing else binds.
  - work is chunked at half-group granularity (4 images) with separate
    tiles per half: short ramp, small tail, smooth pipeline.
"""

import sys

sys.path.insert(0, "/opt/trn_rl_repo")

import numpy as np

N_CORES = 8
H = W = 256
IMG_TOTAL = 4 * 512
IMG_PER_CORE = IMG_TOTAL // N_CORES  # 256
G = 8   # images per group
HG = 4  # images per half-group (one DMA)
NGROUPS = IMG_PER_CORE // G
KH = KW = 4


def _decompose(k):
    """k (4,4) float64 -> list of rank-1 terms [(a[4], taps)], where
    taps = [(scale, [shifts...])]; shift s means column x+s contributes
    with weight scale (after pre-summing all shifts in the group)."""
    U, S, Vt = np.linalg.svd(k)
    terms = []
    for r in range(KH):
        if S[r] <= max(S[0] * 1e-7, 1e-30):
            continue
        a = U[:, r] * np.sqrt(S[r])
        b = Vt[r] * np.sqrt(S[r])
        # tap j has shift 1-j and weight b[j]
        tol = 1e-9 * max(1.0, np.abs(b).max())
        if abs(b[0] - b[3]) <= tol and abs(b[1] - b[2]) <= tol:
            taps = [(b[0], [1, -2]), (b[1], [0, -1])]
        else:
            taps = [(b[j], [1 - j]) for j in range(KW)]
        terms.append((a, taps))
    return terms


def _build_weights(terms):
    """Host-side stationary blocks.  Returns (W_host [128, NIDX, 128] f16,
    mov_shifts: list of shift-lists, one per moving tensor)."""
    movs = []  # (a_vec, scale, shifts)
    for a, taps in terms:
        for scale, shifts in taps:
            movs.append((a, scale, shifts))
    n_idx = len(movs) * 4
    Wh = np.zeros((128, n_idx, 128), np.float32)
    yy = np.arange(H)
    for mi, (a, scale, _shifts) in enumerate(movs):
        V = np.zeros((H, H), np.float64)
        for i in range(KH):
            V[yy, (yy + 1 - i) % H] += a[i]
        VT = (scale * V).T  # VT[v, y]
        for kc in range(2):
            for yb in range(2):
                idx = (mi * 2 + kc) * 2 + yb
                # row v=2*vp+kc lives on partition vp; out row y=2*m+yb on
                # psum partition m (even/odd interleave -> aligned DMA rows)
                Wh[:, idx, :] = VT[kc::2, yb::2].astype(np.float32)
    return Wh.astype(np.float16), [m[2] for m in movs]


_PROGRAM_CACHE = {}


def _build_program(mov_shifts):
    """Build + compile the per-core Bass program.  mov_shifts: list of
    shift-lists (structure only; weights arrive via the `w` input)."""
    import concourse.bacc as bacc
    import concourse.mybir as mybir
    from concourse import tile

    key = tuple(tuple(s) for s in mov_shifts)
    if key in _PROGRAM_CACHE:
        return _PROGRAM_CACHE[key]

    f16 = mybir.dt.float16
    f32 = mybir.dt.float32
    n_movs = len(mov_shifts)
    n_idx = n_movs * 4

    nc = bacc.Bacc("TRN2", target_bir_lowering=False, debug=False,
                   num_devices=N_CORES)
    x_in = nc.declare_dram_parameter("x", [IMG_PER_CORE, H, W], f16,
                                     isOutput=False)
    w_in = nc.declare_dram_parameter("w", [128, n_idx, 128], f16,
                                     isOutput=False)
    y_out = nc.declare_dram_parameter("y", [IMG_PER_CORE, H, W], f16,
                                      isOutput=True)

    # halo layout: xc col c holds image col x = c - 2 for c in [2, 258);
    # cols 0,1,258 are circular-wrap copies, so every shifted window
    # (shifts in [-2, +1]) is contiguous and full-width -- which is what
    # lets the DVE adds run in the 2-elem/cycle fast path
    HW_ = W + 3

    def fill_halo(xc):
        for c, src in ((0, 256), (1, 257), (258, 2)):
            nc.vector.tensor_copy(xc[:, :, :, c:c + 1],
                                  xc[:, :, :, src:src + 1])

    def build_mov(pt, xc, shifts):
        """pt[x] = sum_s xc[(x+s) % W], one full-width op (halo covers
        the wrap)."""
        if len(shifts) == 1:
            s = shifts[0] + 2
            nc.vector.tensor_copy(pt[:], xc[:, :, :, s:s + W])
        else:
            assert len(shifts) == 2
            s0, s1 = shifts[0] + 2, shifts[1] + 2
            nc.vector.tensor_add(
                pt[:], xc[:, :, :, s0:s0 + W], xc[:, :, :, s1:s1 + W]
            )

    with tile.TileContext(nc) as tc:
        with (
            tc.tile_pool(name="const", bufs=1) as cpool,
            tc.tile_pool(name="xin", bufs=6) as xpool,
            tc.tile_pool(name="mov", bufs=3) as mpool,
            tc.tile_pool(name="outp", bufs=4) as opool,
            tc.tile_pool(name="psum", bufs=6, space="PSUM") as pspool,
        ):
            # weight load on the ACT (scalar) queue so the sync queue's
            # first x-load descriptor generation starts immediately; one
            # DMA per idx block so the per-partition descriptors spread
            # over all 16 SDMA engines instead of landing on one
            wt = cpool.tile([128, n_idx, 128], f16)
            for i in range(n_idx):
                nc.scalar.dma_start(wt[:, i, :], w_in[:, i, :])

            for g in range(NGROUPS):
                movs = [[None] * 2 for _ in range(n_movs)]
                yts = [None, None]
                for h in range(2):
                    hs = slice(g * G + h * HG, g * G + (h + 1) * HG)
                    xc = xpool.tile([128, HG, 2, HW_], f16, tag=f"xc{h}",
                                    name=f"xc{h}")
                    # one DMA per row-parity: the padded (stride-259)
                    # tile only AP-balances as [128][img][256] per r
                    # slice; the finer granularity also ramps the SDMA
                    # engines quickly at the head
                    src = x_in[hs].rearrange("m (p r) w -> p m r w", r=2)
                    for rr in range(2):
                        nc.sync.dma_start(
                            xc[:, :, rr, 2:2 + W], src[:, :, rr, :]
                        )
                    fill_halo(xc)
                    for ti, shifts in enumerate(mov_shifts):
                        pt = mpool.tile([128, HG, 2, W], f16,
                                        tag=f"p{ti}h{h}", name=f"p{ti}h{h}")
                        build_mov(pt, xc, shifts)
                        movs[ti][h] = pt
                    yts[h] = opool.tile([128, HG, 2, W], f16, tag=f"yt{h}",
                                        name=f"yt{h}")

                for pr in range(G // 2):
                    h, j = pr // 2, (pr % 2) * 2
                    yt = yts[h]
                    for yb in range(2):
                        ps = pspool.tile([128, 2, W], f32, tag="ps")
                        mms = [(mi, kc) for mi in range(n_movs)
                               for kc in range(2)]
                        for q, (mi, kc) in enumerate(mms):
                            idx = (mi * 2 + kc) * 2 + yb
                            rhs = movs[mi][h][:, j:j + 2, kc, 0:W]
                            nc.tensor.matmul(
                                ps[:], wt[:, idx, :], rhs,
                                start=(q == 0), stop=(q == len(mms) - 1),
                            )
                        # psum -> fp16 staging: split 6/2 between ACT and
                        # DVE so neither binds (the tensor engine is the
                        # intended bottleneck)
                        dst = yt[:, j:j + 2, yb, :]
                        if pr == 3 and yb == 0 or pr == 1 and yb == 0:
                            nc.vector.tensor_copy(dst, ps[:])
                        else:
                            nc.scalar.copy(dst, ps[:])
                    if pr % 2 == 1:
                        # store right after the producing copies; h0's
                        # descriptor generation on the Sync queue, h1's on
                        # the ACT queue, so neither sequencer carries the
                        # whole store-generation load
                        hs = slice(g * G + h * HG, g * G + (h + 1) * HG)
                        dma_eng = nc.sync if h == 0 else nc.scalar
                        dma_eng.dma_start(
                            y_out[hs].rearrange("m (p r) w -> p m r w", r=2),
                            yt[:],
                        )

    nc.compile()
    _PROGRAM_CACHE[key] = nc
    return nc


def _make_in_maps(input_np, Wh):
    """Per-core input maps.  The host ships x as fp16 (the rel-err budget
    is 2e-2; fp16 I/O quantization costs ~3e-4) to halve HBM traffic."""
    x16 = np.ascontiguousarray(
        np.asarray(input_np).reshape(IMG_TOTAL, H, W).astype(np.float16)
    )
    return [
        {"x": x16[c * IMG_PER_CORE:(c + 1) * IMG_PER_CORE], "w": Wh}
        for c in range(N_CORES)
    ]


def kernel(input, kernel):
    input = np.asarray(input, dtype=np.float32)
    k = np.asarray(kernel, dtype=np.float64)
    assert input.shape == (4, 512, H, W) and k.shape == (KH, KW)

    terms = _decompose(k)
    if not terms:
        return np.zeros_like(input)

    Wh, mov_shifts = _build_weights(terms)
    nc = _build_program(mov_shifts)

    from concourse.bass_utils import run_bass_kernel_spmd

    in_maps = _make_in_maps(input, Wh)
    res = run_bass_kernel_spmd(nc, in_maps, list(range(N_CORES)))
    out = np.concatenate([res.results[c]["y"] for c in range(N_CORES)], axis=0)
    return out.reshape(4, 512, H, W).astype(np.float32)
